# revision 45
# baseline (speedup 1.0000x reference)
"""Trainium2 Bass kernel for GQA attention (nn_Attention_15015205667492).

Reference computation (per batch b, seq s=2048, d=2048):
  q = (x @ wq)  -> 32 heads x 64     (RoPE)
  k = (x @ wk)  ->  8 kv heads x 64  (RoPE)
  v = (x @ wv)  ->  8 kv heads x 64
  causal softmax(q k^T / 8) @ v  (GQA: kv head = q head // 4)
  out = attn @ wo

Sharding (8 cores): DP2 x TP4.
  core c: batch = c//4, head-group g = c%4 (Q heads 8g..8g+7, KV heads 2g, 2g+1).
  Each core computes attention for its 8 heads over its batch, writes the
  head-transposed attention output [512, S] bf16 to DRAM, AllGathers it within
  its 4-core batch group -> [2048, S], then computes a column slice of o_proj
  (wo[:, 512g:512(g+1)]) so per-core outputs are disjoint blocks of the final
  output (host-side unshard is pure concatenation).

v2 schedule: one long interleaved emission stream so the per-engine static
order keeps TensorE dense: V proj, K proj, Q proj(cols 0:1024), then the
attention query-blocks with Q proj(cols 1024:2048) quanta injected into
qb0/qb1 and o_proj quanta (per 128-row slice) injected into qb2/qb3 as each
AllGather lands.  Scores/exp/AV use a [128, 2, 512] layout; the causal mask
multiply is a [128,2,128] op on the first 128 cols of each block's visible
window (blocks left of the diagonal shrink their window by 128*j).  The
softmax denominator rides row 64 of the AV psum (ones column in V); the
colsum broadcast matmuls run in bf16.  PSUM drains run on whichever of
ScalarE/VectorE is off the critical path (exp lives on ScalarE).
"""

import sys

sys.path.insert(0, "/opt/trn_rl_repo")

import numpy as np
import ml_dtypes

N_CORES = 8
H, KVH, HD = 32, 8, 64
RG = [[0, 1, 2, 3], [4, 5, 6, 7]]

_cache = {}


def build_program(S=2048, D=2048, enable_asserts=False, NO_CC=False, bench_iters=0):
    import concourse.mybir as mybir
    import concourse.tile as tile
    from concourse import bacc

    f32 = mybir.dt.float32
    bf16 = mybir.dt.bfloat16
    Exp = mybir.ActivationFunctionType.Exp

    DC = D // 128       # contraction chunks for projections
    QB = S // 512       # query blocks (512 q rows each)
    KB = S // 128       # key blocks
    DOUT = D // 4       # output column slice per core
    HC = (H * HD) // 128  # o_proj contraction chunks (16)

    nc = bacc.Bacc(
        "TRN2",
        target_bir_lowering=False,
        debug=False,
        enable_asserts=enable_asserts,
        num_devices=N_CORES,
    )

    xT_d = nc.dram_tensor("xT", [DC, 128, S], bf16, kind="ExternalInput")
    wq_d = nc.dram_tensor("wq", [DC, 128, 512], bf16, kind="ExternalInput")
    wk_d = nc.dram_tensor("wk", [DC, 128, 128], bf16, kind="ExternalInput")
    wv_d = nc.dram_tensor("wv", [DC, 128, 128], bf16, kind="ExternalInput")
    wo_d = nc.dram_tensor("wo", [H * HD, DOUT], bf16, kind="ExternalInput")
    cos_d = nc.dram_tensor("cos2", [128, S], bf16, kind="ExternalInput")
    sin_d = nc.dram_tensor("sinsw2", [128, S], bf16, kind="ExternalInput")
    rot_d = nc.dram_tensor("rot", [128, 128], bf16, kind="ExternalInput")
    msk_d = nc.dram_tensor("masks", [128, 128], bf16, kind="ExternalInput")
    out_d = nc.dram_tensor("out", [S, DOUT], f32, kind="ExternalOutput")

    with tile.TileContext(nc) as tc:
        with (
            tc.tile_pool(name="const", bufs=1) as const,
            tc.tile_pool(name="psA", bufs=2, space="PSUM") as psA,
            tc.tile_pool(name="psAV", bufs=2, space="PSUM") as psAV,
            tc.tile_pool(name="psP", bufs=2, space="PSUM") as psP,
            tc.tile_pool(name="work", bufs=2) as work,
            tc.tile_pool(name="dram", bufs=1, space="DRAM") as dram,
        ):
            # ---------------- constants / weights ----------------
            xt8 = const.tile([128, DC, S], bf16, name="xt8", tag="xt8")
            wq8 = const.tile([128, DC, 512], bf16, name="wq8", tag="wq8")
            wk8 = const.tile([128, DC, 128], bf16, name="wk8", tag="wk8")
            wv8 = const.tile([128, DC, 128], bf16, name="wv8", tag="wv8")
            # window-major x loads so the first seq-window's projections can
            # start after ~1/4 of the x DMA instead of all of it
            NW = S // 512
            for w in range(NW):
                for i in range(DC):
                    nc.sync.dma_start(
                        out=xt8[:, i, 512 * w : 512 * (w + 1)],
                        in_=xT_d[i, :, 512 * w : 512 * (w + 1)],
                    )
                if w == 0:
                    for i in range(DC):
                        nc.sync.dma_start(out=wk8[:, i, :], in_=wk_d[i, :, :])
                        nc.sync.dma_start(out=wv8[:, i, :], in_=wv_d[i, :, :])
                if w == 0:
                    cos_sb = const.tile([128, S], bf16, name="cos", tag="cos")
                    nc.sync.dma_start(out=cos_sb[:], in_=cos_d[:, :])
                    sin_sb = const.tile([128, S], bf16, name="sin", tag="sin")
                    nc.sync.dma_start(out=sin_sb[:], in_=sin_d[:, :])
                    rot_sb = const.tile([128, 128], bf16, name="rot", tag="rot")
                    nc.sync.dma_start(out=rot_sb[:], in_=rot_d[:, :])
                    msk_sb = const.tile([128, 128], bf16, name="msk", tag="msk")
                    nc.sync.dma_start(out=msk_sb[:], in_=msk_d[:, :])
                if w == 1:
                    for i in range(DC):
                        nc.sync.dma_start(out=wq8[:, i, :], in_=wq_d[i, :, :])
            wo_t = []
            for i in range(HC):
                t = const.tile([128, DOUT], bf16, name=f"wo{i}", tag=f"wo{i}")
                nc.sync.dma_start(out=t[:], in_=wo_d[128 * i : 128 * (i + 1), :])
                wo_t.append(t)
            ones_sb = const.tile([65, 64], bf16, name="ones", tag="ones")
            nc.vector.memset(ones_sb[:], 1.0)

            def emit_body():
                CH = min(1024, S)
                NC2 = S // CH

                qT = [
                    const.tile([128, S], bf16, name=f"qT{p}", tag=f"qT{p}")
                    for p in range(4)
                ]
                kTd = [
                    const.tile([128, S], bf16, name=f"kTd{h}", tag=f"kTd{h}")
                    for h in range(2)
                ]
                v_sb = [
                    const.tile([128, 132], bf16, name=f"v{kb}", tag=f"v{kb}")
                    for kb in range(KB)
                ]
                cc_in = [
                    dram.tile([512, 512], bf16, name=f"cin{qb}", tag=f"cin{qb}")
                    for qb in range(QB - 1)
                ]
                cc_out = [
                    dram.tile([2048, 512], bf16, name=f"cout{qb}", tag=f"cout{qb}")
                    for qb in range(QB - 1)
                ]
                # last query block gathers per pair so o_proj's tail only
                # waits on the final pair's collective
                cc3_in = [
                    dram.tile([128, 512], bf16, name=f"cin3p{p}", tag=f"cin3p{p}")
                    for p in range(4)
                ]
                cc3_out = [
                    dram.tile([512, 512], bf16, name=f"cout3p{p}", tag=f"cout3p{p}")
                    for p in range(4)
                ]

                # ---------------- projections + RoPE ----------------
                # RoPE in T-layout: rows = hd index (2 heads stacked), cols =
                # seq; rot-half is a PE permutation (sign folded into sinsw2).
                def proj_rope(w_tiles, col0, dest, c2, dest_split=None):
                    raw = work.tile([128, CH], bf16, name="raw", tag="raw", bufs=2)
                    tmp = work.tile([128, CH], bf16, name="ropetmp", tag="ropetmp", bufs=2)
                    pq = [
                        psP.tile([128, 512], f32, name=f"pq{q2}", tag="pp")
                        for q2 in range(2)
                    ]
                    # sequential windows: window q2=0 runs dense even while
                    # window q2=1's x chunk is still streaming in
                    for q2 in range(2):
                        for dc in range(DC):
                            qc = 2 * c2 + q2
                            nc.tensor.matmul(
                                pq[q2][:],
                                w_tiles[:, dc, col0 : col0 + 128],
                                xt8[:, dc, 512 * qc : 512 * (qc + 1)],
                                start=(dc == 0),
                                stop=(dc == DC - 1),
                                skip_group_check=True,
                            )
                    for q2 in range(2):
                        nc.any.tensor_copy(
                            out=raw[:, 512 * q2 : 512 * (q2 + 1)], in_=pq[q2][:]
                        )
                    for q2 in range(2):
                        pr = psP.tile([128, 512], f32, name="pr", tag="pp")
                        nc.tensor.matmul(
                            pr[:],
                            rot_sb[:],
                            raw[:, 512 * q2 : 512 * (q2 + 1)],
                            start=True,
                            stop=True,
                        )
                        nc.vector.tensor_mul(
                            tmp[:, 512 * q2 : 512 * (q2 + 1)],
                            pr[:],
                            sin_sb[:, CH * c2 + 512 * q2 : CH * c2 + 512 * (q2 + 1)],
                        )
                    nc.vector.tensor_mul(
                        raw[:], raw[:], cos_sb[:, CH * c2 : CH * (c2 + 1)]
                    )
                    if dest_split is None:
                        nc.vector.tensor_add(
                            dest[:, CH * c2 : CH * (c2 + 1)], raw[:], tmp[:]
                        )
                    else:
                        # K proj: head h's dims (rows 64h:64h+64) land in the
                        # top half of kTd[h]; the bottom half is a dup (DMA'd)
                        for h in range(2):
                            nc.vector.tensor_add(
                                dest_split[h][0:64, CH * c2 : CH * (c2 + 1)],
                                raw[64 * h : 64 * h + 64, :],
                                tmp[64 * h : 64 * h + 64, :],
                            )

                def vproj(kb):
                    vt = v_sb[kb]
                    nc.vector.memset(vt[:, 64:65], 1.0)
                    nc.vector.memset(vt[:, 129:130], 1.0)
                    pv = psP.tile([128, 128], f32, name="pv", tag="pp")
                    for dc in range(DC):
                        nc.tensor.matmul(
                            pv[:],
                            xt8[:, dc, 128 * kb : 128 * (kb + 1)],
                            wv8[:, dc, :],
                            start=(dc == 0),
                            stop=(dc == DC - 1),
                        )
                    nc.any.tensor_copy(out=vt[:, 0:64], in_=pv[:, 0:64])
                    nc.any.tensor_copy(out=vt[:, 65:129], in_=pv[:, 64:128])

                # ---------------- attention ----------------
                def attn_pair(qb, pidx):
                    hg = pidx // 2
                    kmax = 4 * (qb + 1)
                    # one accumulator bank per head so each drains (and frees
                    # its slot for the next pair) independently
                    pav = [
                        psAV.tile([65, 512], f32, name=f"pav{i}", tag="pav")
                        for i in range(2)
                    ]

                    def emit_av(kb, pt, vw):
                        for i in range(2):
                            nc.tensor.matmul(
                                pav[i][:, 512 - vw : 512],
                                v_sb[kb][:, 65 * hg : 65 * hg + 65],
                                pt[:, i, 0:vw],
                                start=(kb == 0),
                                stop=(kb == kmax - 1),
                            )

                    pend = None  # software-pipeline: AV lags scores/exp by one
                    for kb in range(kmax):
                        j = kb - 4 * qb
                        vw = 512 - 128 * j if j >= 0 else 512
                        q0 = 512 * qb + (512 - vw)
                        ps = psA.tile([128, 2, 512], f32, name="ps", tag="ps")
                        for i in range(2):
                            r0 = 64 * i
                            nc.tensor.matmul(
                                ps[:, i, 0:vw],
                                kTd[hg][r0 : r0 + 64, 128 * kb : 128 * (kb + 1)],
                                qT[pidx][r0 : r0 + 64, q0 : q0 + vw],
                                start=True,
                                stop=True,
                            )
                        pt = work.tile([128, 2, 512], bf16, name="pt", tag="pt", bufs=5)
                        nc.scalar.activation(
                            out=pt[:, :, 0:vw], in_=ps[:, :, 0:vw], func=Exp, scale=0.125
                        )
                        if j >= 0:
                            # only the first 128 cols of the visible window are
                            # on the diagonal; the rest is fully visible
                            for i in range(2):
                                nc.vector.tensor_mul(
                                    pt[:, i, 0:128], pt[:, i, 0:128], msk_sb[:, 0:128]
                                )
                        emit_av(kb, pt, vw)
                    # normalize: out = num * (1/colsum), colsum broadcast via PE
                    ou = work.tile([65, 1024], bf16, name="ou", tag="ou", bufs=2)
                    for i in range(2):
                        nc.any.tensor_copy(
                            out=ou[:, 512 * i : 512 * (i + 1)], in_=pav[i][:]
                        )
                    at = work.tile([64, 1024], bf16, name="at", tag="at")
                    for i in range(2):
                        pb = psP.tile([64, 512], f32, name=f"pb{i}", tag="pp")
                        nc.tensor.matmul(
                            pb[:],
                            ones_sb[64:65, :],
                            ou[64:65, 512 * i : 512 * (i + 1)],
                            start=True,
                            stop=True,
                        )
                        rbc = work.tile([64, 512], f32, name="rbc", tag="rbc", bufs=2)
                        nc.vector.reciprocal_approx_fast(out=rbc[:], in_=pb[:])
                        # pure-SBUF multiply: run on the otherwise-idle GpSimd
                        # so VectorE (PSUM drains, masks, rope) stops gating PE
                        nc.gpsimd.tensor_mul(
                            at[:, 512 * i : 512 * (i + 1)],
                            ou[0:64, 512 * i : 512 * (i + 1)],
                            rbc[:],
                        )
                    for i in range(2):
                        if qb == QB - 1:
                            dst = cc3_in[pidx][64 * i : 64 * (i + 1), :]
                        else:
                            dst = cc_in[qb][
                                128 * pidx + 64 * i : 128 * pidx + 64 * (i + 1), :
                            ]
                        nc.sync.dma_start(
                            out=dst, in_=at[:, 512 * i : 512 * (i + 1)]
                        )

                def _ag(tin, tout, rows):
                    if NO_CC:
                        nc.sync.dma_start(out=tout[0:rows, :], in_=tin[:, :])
                    else:
                        nc.gpsimd.collective_compute(
                            "AllGather",
                            mybir.AluOpType.bypass,
                            replica_groups=RG,
                            ins=[tin.opt()],
                            outs=[tout.opt()],
                        )

                def allgather(qb):
                    _ag(cc_in[qb], cc_out[qb], 512)

                # ---------------- o_proj quanta ----------------
                def oproj_quanta(qb):
                    cct = {}
                    last = qb == QB - 1
                    # for the per-pair-gathered last block, order contraction
                    # chunks pair-major so pair 3's chunks accumulate last
                    hc_order = (
                        [4 * c + p for p in range(4) for c in range(4)]
                        if last
                        else list(range(HC))
                    )

                    def load(lo, hi):
                        def _f():
                            for hc in hc_order[lo:hi]:
                                t = work.tile(
                                    [128, 512], bf16, name=f"cct{hc}", tag=f"cct{hc}",
                                    bufs=2,
                                )
                                if last:
                                    c, p = divmod(hc, 4)
                                    src = cc3_out[p][128 * c : 128 * (c + 1), :]
                                else:
                                    src = cc_out[qb][128 * hc : 128 * (hc + 1), :]
                                nc.sync.dma_start(out=t[:], in_=src)
                                cct[hc] = t
                        return _f

                    def rb_quant(rb):
                        def _f():
                            po = psP.tile([128, DOUT], f32, name="po", tag="pp")
                            for n, hc in enumerate(hc_order):
                                nc.tensor.matmul(
                                    po[:],
                                    cct[hc][:, 128 * rb : 128 * (rb + 1)],
                                    wo_t[hc][:],
                                    start=(n == 0),
                                    stop=(n == HC - 1),
                                )
                            ot = work.tile([128, DOUT], f32, name="ot", tag="ot", bufs=2)
                            nc.any.tensor_copy(out=ot[:], in_=po[:])
                            nc.sync.dma_start(
                                out=out_d[
                                    512 * qb + 128 * rb : 512 * qb + 128 * (rb + 1), :
                                ],
                                in_=ot[:],
                            )
                        return _f

                    return [load(0, 8), load(8, 16)] + [rb_quant(rb) for rb in range(4)]

                # ---------------- the interleaved schedule ----------------
                # PE stream follows x-window DMA arrival: w0/w1 work first,
                # w2/w3-dependent V blocks and K c2=1 after
                for kb in range(8):
                    vproj(kb)
                proj_rope(wk8, 0, None, 0, dest_split=kTd)
                # dup each K chunk right away so the copy overlaps later work
                for h in range(2):
                    nc.sync.dma_start(
                        out=kTd[h][64:128, 0:CH], in_=kTd[h][0:64, 0:CH]
                    )
                for p in range(4):
                    proj_rope(wq8, 128 * p, qT[p], 0)
                for kb in range(8, 12):
                    vproj(kb)
                proj_rope(wk8, 0, None, 1, dest_split=kTd)
                for h in range(2):
                    nc.sync.dma_start(
                        out=kTd[h][64:128, CH : 2 * CH], in_=kTd[h][0:64, CH : 2 * CH]
                    )
                for kb in range(12, KB):
                    vproj(kb)

                oq = {}
                # injections: (qb, pidx) -> list of emission closures
                inj = {
                    (0, 1): [lambda: proj_rope(wq8, 0, qT[0], 1)],
                    (0, 3): [lambda: proj_rope(wq8, 128, qT[1], 1)],
                    (1, 1): [lambda: proj_rope(wq8, 256, qT[2], 1)],
                    (1, 3): [lambda: proj_rope(wq8, 384, qT[3], 1)],
                }

                for qb in range(QB):
                    if qb == 2:
                        oq[0] = oproj_quanta(0)
                        inj[(2, 0)] = [oq[0][0]]
                        inj[(2, 1)] = [oq[0][1], oq[0][2]]
                        inj[(2, 2)] = [oq[0][3], oq[0][4]]
                        inj[(2, 3)] = [oq[0][5]]
                    if qb == 3:
                        oq[1] = oproj_quanta(1)
                        oq[2] = oproj_quanta(2)
                        inj[(3, 0)] = oq[1][0:3]
                        inj[(3, 1)] = oq[1][3:6]
                        inj[(3, 2)] = oq[2][0:3]
                        # keep two oproj(2) quanta for the tail so PE has
                        # AG3p3-independent work while that gather is in flight
                        inj[(3, 3)] = oq[2][3:4]
                    for pidx in range(4):
                        attn_pair(qb, pidx)
                        if qb == QB - 1:
                            _ag(cc3_in[pidx], cc3_out[pidx], 128)
                        for f in inj.get((qb, pidx), []):
                            f()
                    if qb < QB - 1:
                        allgather(qb)
                oq3 = oproj_quanta(3)
                # tail order: finish oproj(2) first so its cct slots free up
                # for qb3's loads, which then overlap the AG3p3 wait
                for f in oq[2][4:6] + oq3:
                    f()

            if bench_iters:
                with tc.For_i(0, bench_iters, 1, name="bench"):
                    emit_body()
            else:
                emit_body()

    nc.compile()
    return nc


def prep_inputs(x, cos, sin, wq, wk, wv, wo):
    """Shard + reformat full inputs into per-core input maps.

    (fp8 projections were tried and reverted: attention output is a
    random-walk sum over keys, so e4m3's ~2.6% RMS quantization error lands
    ~1:1 on the output -- 2.8-5.4% rel err, over the 2e-2 gate.)
    """
    bf = ml_dtypes.bfloat16
    b, s, d = x.shape
    dout = d // 4
    cos2 = np.tile(np.ascontiguousarray(cos.T), (2, 1)).astype(bf)
    sinT = np.ascontiguousarray(sin.T)
    sinsw = np.concatenate([-sinT[:32], sinT[32:]], axis=0)
    sinsw2 = np.tile(sinsw, (2, 1)).astype(bf)
    # rotate-half permutation: tmp[i] = raw[sigma(i)]; out = R.T @ raw
    rotm = np.zeros((128, 128), np.float32)
    for i in range(128):
        j = (i // 64) * 64 + ((i % 64) + 32) % 64
        rotm[j, i] = 1.0
    rotm = rotm.astype(bf)
    k_loc = np.arange(128)[:, None]
    q_loc = np.arange(128)[None, :]
    masks = (k_loc <= q_loc).astype(np.float32).astype(bf)  # [128,128]

    dc = d // 128
    in_maps = []
    for c in range(N_CORES):
        bb, g = divmod(c, 4)
        in_maps.append(
            {
                "xT": np.ascontiguousarray(x[bb].T.reshape(dc, 128, s)).astype(bf),
                "wq": np.ascontiguousarray(
                    wq[:, 512 * g : 512 * (g + 1)].reshape(dc, 128, 512)
                ).astype(bf),
                "wk": np.ascontiguousarray(
                    wk[:, 128 * g : 128 * (g + 1)].reshape(dc, 128, 128)
                ).astype(bf),
                "wv": np.ascontiguousarray(
                    wv[:, 128 * g : 128 * (g + 1)].reshape(dc, 128, 128)
                ).astype(bf),
                "wo": np.ascontiguousarray(wo[:, dout * g : dout * (g + 1)]).astype(bf),
                "cos2": cos2,
                "sinsw2": sinsw2,
                "rot": rotm,
                "masks": masks,
            }
        )
    return in_maps


def assemble_output(results, b, s, d):
    full = np.empty((b, s, d), np.float32)
    dout = d // 4
    for c in range(N_CORES):
        bb, g = divmod(c, 4)
        full[bb][:, dout * g : dout * (g + 1)] = results[c]["out"]
    return full


def kernel(**inputs):
    x = np.asarray(inputs["x"], np.float32)
    b, s, d = x.shape
    key = (s, d)
    if key not in _cache:
        _cache[key] = build_program(S=s, D=d)
    nc = _cache[key]
    in_maps = prep_inputs(
        x,
        np.asarray(inputs["cos"], np.float32),
        np.asarray(inputs["sin"], np.float32),
        np.asarray(inputs["wq"], np.float32),
        np.asarray(inputs["wk"], np.float32),
        np.asarray(inputs["wv"], np.float32),
        np.asarray(inputs["wo"], np.float32),
    )
    from concourse.bass_utils import run_bass_kernel_spmd

    res = run_bass_kernel_spmd(nc, in_maps, core_ids=list(range(N_CORES)))
    return assemble_output(res.results, b, s, d)


# revision 46
# speedup vs baseline: 1.1966x; 1.1966x over previous
"""Trainium2 Bass kernel for GQA attention (nn_Attention_15015205667492).

Reference computation (per batch b, seq s=2048, d=2048):
  q = (x @ wq)  -> 32 heads x 64     (RoPE)
  k = (x @ wk)  ->  8 kv heads x 64  (RoPE)
  v = (x @ wv)  ->  8 kv heads x 64
  causal softmax(q k^T / 8) @ v  (GQA: kv head = q head // 4)
  out = attn @ wo

Sharding (8 cores): DP2 x TP4.
  core c: batch = c//4, head-group g = c%4 (Q heads 8g..8g+7, KV heads 2g, 2g+1).
  Each core computes attention for its 8 heads over its batch, writes the
  head-transposed attention output [512, S] bf16 to DRAM, AllGathers it within
  its 4-core batch group -> [2048, S], then computes a column slice of o_proj
  (wo[:, 512g:512(g+1)]) so per-core outputs are disjoint blocks of the final
  output (host-side unshard is pure concatenation).

v2 schedule: one long interleaved emission stream so the per-engine static
order keeps TensorE dense: V proj, K proj, Q proj(cols 0:1024), then the
attention query-blocks with Q proj(cols 1024:2048) quanta injected into
qb0/qb1 and o_proj quanta (per 128-row slice) injected into qb2/qb3 as each
AllGather lands.  Scores/exp/AV use a [128, 2, 512] layout; the causal mask
multiply is a [128,2,128] op on the first 128 cols of each block's visible
window (blocks left of the diagonal shrink their window by 128*j).  The
softmax denominator rides row 64 of the AV psum (ones column in V); the
colsum broadcast matmuls run in bf16.  PSUM drains run on whichever of
ScalarE/VectorE is off the critical path (exp lives on ScalarE).
"""

import sys

sys.path.insert(0, "/opt/trn_rl_repo")

import numpy as np
import ml_dtypes

N_CORES = 8
H, KVH, HD = 32, 8, 64
RG = [[0, 1, 2, 3], [4, 5, 6, 7]]

_cache = {}


def build_program(S=2048, D=2048, enable_asserts=False, NO_CC=False, bench_iters=0):
    import concourse.mybir as mybir
    import concourse.tile as tile
    from concourse import bacc

    f32 = mybir.dt.float32
    bf16 = mybir.dt.bfloat16
    Exp = mybir.ActivationFunctionType.Exp

    DC = D // 128       # contraction chunks for projections
    QB = S // 512       # query blocks (512 q rows each)
    KB = S // 128       # key blocks
    DOUT = D // 4       # output column slice per core
    HC = (H * HD) // 128  # o_proj contraction chunks (16)

    nc = bacc.Bacc(
        "TRN2",
        target_bir_lowering=False,
        debug=False,
        enable_asserts=enable_asserts,
        num_devices=N_CORES,
    )

    xT_d = nc.dram_tensor("xT", [DC, 128, S], bf16, kind="ExternalInput")
    wq_d = nc.dram_tensor("wq", [DC, 128, 512], bf16, kind="ExternalInput")
    wk_d = nc.dram_tensor("wk", [DC, 128, 128], bf16, kind="ExternalInput")
    wv_d = nc.dram_tensor("wv", [DC, 128, 128], bf16, kind="ExternalInput")
    wo_d = nc.dram_tensor("wo", [H * HD, DOUT], bf16, kind="ExternalInput")
    cos_d = nc.dram_tensor("cos2", [128, S], bf16, kind="ExternalInput")
    sin_d = nc.dram_tensor("sinsw2", [128, S], bf16, kind="ExternalInput")
    rot_d = nc.dram_tensor("rot", [128, 128], bf16, kind="ExternalInput")
    msk_d = nc.dram_tensor("masks", [128, 128], bf16, kind="ExternalInput")
    out_d = nc.dram_tensor("out", [S, DOUT], f32, kind="ExternalOutput")

    with tile.TileContext(nc) as tc:
        with (
            tc.tile_pool(name="const", bufs=1) as const,
            tc.tile_pool(name="psA", bufs=2, space="PSUM") as psA,
            tc.tile_pool(name="psAV", bufs=2, space="PSUM") as psAV,
            tc.tile_pool(name="psP", bufs=2, space="PSUM") as psP,
            tc.tile_pool(name="work", bufs=2) as work,
            tc.tile_pool(name="dram", bufs=1, space="DRAM") as dram,
        ):
            # ---------------- constants / weights ----------------
            xt8 = const.tile([128, DC, S], bf16, name="xt8", tag="xt8")
            wq8 = const.tile([128, DC, 512], bf16, name="wq8", tag="wq8")
            wk8 = const.tile([128, DC, 128], bf16, name="wk8", tag="wk8")
            wv8 = const.tile([128, DC, 128], bf16, name="wv8", tag="wv8")
            # window-major x loads so the first seq-window's projections can
            # start after ~1/4 of the x DMA instead of all of it
            NW = S // 512
            for w in range(NW):
                for i in range(DC):
                    nc.sync.dma_start(
                        out=xt8[:, i, 512 * w : 512 * (w + 1)],
                        in_=xT_d[i, :, 512 * w : 512 * (w + 1)],
                    )
                if w == 0:
                    for i in range(DC):
                        nc.sync.dma_start(out=wk8[:, i, :], in_=wk_d[i, :, :])
                        nc.sync.dma_start(out=wv8[:, i, :], in_=wv_d[i, :, :])
                if w == 0:
                    cos_sb = const.tile([128, S], bf16, name="cos", tag="cos")
                    nc.sync.dma_start(out=cos_sb[:], in_=cos_d[:, :])
                    sin_sb = const.tile([128, S], bf16, name="sin", tag="sin")
                    nc.sync.dma_start(out=sin_sb[:], in_=sin_d[:, :])
                    rot_sb = const.tile([128, 128], bf16, name="rot", tag="rot")
                    nc.sync.dma_start(out=rot_sb[:], in_=rot_d[:, :])
                    msk_sb = const.tile([128, 128], bf16, name="msk", tag="msk")
                    nc.sync.dma_start(out=msk_sb[:], in_=msk_d[:, :])
                if w == 1:
                    for i in range(DC):
                        nc.sync.dma_start(out=wq8[:, i, :], in_=wq_d[i, :, :])
            wo_t = []
            for i in range(HC):
                t = const.tile([128, DOUT], bf16, name=f"wo{i}", tag=f"wo{i}")
                nc.sync.dma_start(out=t[:], in_=wo_d[128 * i : 128 * (i + 1), :])
                wo_t.append(t)
            ones_sb = const.tile([65, 64], bf16, name="ones", tag="ones")
            nc.vector.memset(ones_sb[:], 1.0)
            v_sb = [
                const.tile([128, 132], bf16, name=f"v{kb}", tag=f"v{kb}")
                for kb in range(S // 128)
            ]
            for kb in range(S // 128):
                nc.vector.memset(v_sb[kb][:, 64:65], 1.0)
                nc.vector.memset(v_sb[kb][:, 129:130], 1.0)

            def emit_body():
                CH = min(1024, S)
                NC2 = S // CH

                qT = [
                    const.tile([128, S], bf16, name=f"qT{p}", tag=f"qT{p}")
                    for p in range(4)
                ]
                kTd = [
                    const.tile([128, S], bf16, name=f"kTd{h}", tag=f"kTd{h}")
                    for h in range(2)
                ]
                cc_in = [
                    dram.tile([512, 512], bf16, name=f"cin{qb}", tag=f"cin{qb}")
                    for qb in range(QB - 1)
                ]
                cc_out = [
                    dram.tile([2048, 512], bf16, name=f"cout{qb}", tag=f"cout{qb}")
                    for qb in range(QB - 1)
                ]
                # last query block gathers per pair so o_proj's tail only
                # waits on the final pair's collective
                cc3_in = [
                    dram.tile([128, 512], bf16, name=f"cin3p{p}", tag=f"cin3p{p}")
                    for p in range(4)
                ]
                cc3_out = [
                    dram.tile([512, 512], bf16, name=f"cout3p{p}", tag=f"cout3p{p}")
                    for p in range(4)
                ]

                # ---------------- projections + RoPE ----------------
                # RoPE in T-layout: rows = hd index (2 heads stacked), cols =
                # seq; rot-half is a PE permutation (sign folded into sinsw2).
                def proj_rope(w_tiles, col0, dest, c2, dest_split=None):
                    raw = work.tile([128, CH], bf16, name="raw", tag="raw", bufs=2)
                    tmp = work.tile([128, CH], bf16, name="ropetmp", tag="ropetmp", bufs=2)
                    pq = [
                        psP.tile([128, 512], f32, name=f"pq{q2}", tag="pp")
                        for q2 in range(2)
                    ]
                    # sequential windows: window q2=0 runs dense even while
                    # window q2=1's x chunk is still streaming in
                    for q2 in range(2):
                        for dc in range(DC):
                            qc = 2 * c2 + q2
                            nc.tensor.matmul(
                                pq[q2][:],
                                w_tiles[:, dc, col0 : col0 + 128],
                                xt8[:, dc, 512 * qc : 512 * (qc + 1)],
                                start=(dc == 0),
                                stop=(dc == DC - 1),
                                skip_group_check=True,
                            )
                    for q2 in range(2):
                        nc.any.tensor_copy(
                            out=raw[:, 512 * q2 : 512 * (q2 + 1)], in_=pq[q2][:]
                        )
                    for q2 in range(2):
                        pr = psP.tile([128, 512], f32, name="pr", tag="pp")
                        nc.tensor.matmul(
                            pr[:],
                            rot_sb[:],
                            raw[:, 512 * q2 : 512 * (q2 + 1)],
                            start=True,
                            stop=True,
                        )
                        nc.vector.tensor_mul(
                            tmp[:, 512 * q2 : 512 * (q2 + 1)],
                            pr[:],
                            sin_sb[:, CH * c2 + 512 * q2 : CH * c2 + 512 * (q2 + 1)],
                        )
                    nc.vector.tensor_mul(
                        raw[:], raw[:], cos_sb[:, CH * c2 : CH * (c2 + 1)]
                    )
                    if dest_split is None:
                        nc.vector.tensor_add(
                            dest[:, CH * c2 : CH * (c2 + 1)], raw[:], tmp[:]
                        )
                    else:
                        # K proj: head h's dims (rows 64h:64h+64) land in the
                        # top half of kTd[h]; the bottom half is a dup (DMA'd)
                        for h in range(2):
                            nc.vector.tensor_add(
                                dest_split[h][0:64, CH * c2 : CH * (c2 + 1)],
                                raw[64 * h : 64 * h + 64, :],
                                tmp[64 * h : 64 * h + 64, :],
                            )

                def vproj(kb):
                    vt = v_sb[kb]
                    pv = psP.tile([128, 128], f32, name="pv", tag="pp")
                    for dc in range(DC):
                        nc.tensor.matmul(
                            pv[:],
                            xt8[:, dc, 128 * kb : 128 * (kb + 1)],
                            wv8[:, dc, :],
                            start=(dc == 0),
                            stop=(dc == DC - 1),
                        )
                    nc.any.tensor_copy(out=vt[:, 0:64], in_=pv[:, 0:64])
                    nc.any.tensor_copy(out=vt[:, 65:129], in_=pv[:, 64:128])

                # ---------------- attention ----------------
                def attn_pair(qb, pidx):
                    hg = pidx // 2
                    kmax = 4 * (qb + 1)
                    # one accumulator bank per head so each drains (and frees
                    # its slot for the next pair) independently
                    pav = [
                        psAV.tile([65, 512], f32, name=f"pav{i}", tag="pav")
                        for i in range(2)
                    ]

                    def emit_av(kb, pt, vw):
                        for i in range(2):
                            nc.tensor.matmul(
                                pav[i][:, 512 - vw : 512],
                                v_sb[kb][:, 65 * hg : 65 * hg + 65],
                                pt[:, i, 0:vw],
                                start=(kb == 0),
                                stop=(kb == kmax - 1),
                            )

                    pend = None  # software-pipeline: AV lags scores/exp by one
                    for kb in range(kmax):
                        j = kb - 4 * qb
                        vw = 512 - 128 * j if j >= 0 else 512
                        q0 = 512 * qb + (512 - vw)
                        ps = psA.tile([128, 2, 512], f32, name="ps", tag="ps")
                        for i in range(2):
                            r0 = 64 * i
                            nc.tensor.matmul(
                                ps[:, i, 0:vw],
                                kTd[hg][r0 : r0 + 64, 128 * kb : 128 * (kb + 1)],
                                qT[pidx][r0 : r0 + 64, q0 : q0 + vw],
                                start=True,
                                stop=True,
                            )
                        pt = work.tile([128, 2, 512], bf16, name="pt", tag="pt", bufs=5)
                        nc.scalar.activation(
                            out=pt[:, :, 0:vw], in_=ps[:, :, 0:vw], func=Exp, scale=0.125
                        )
                        if j >= 0:
                            # only the first 128 cols of the visible window are
                            # on the diagonal; the rest is fully visible
                            for i in range(2):
                                nc.vector.tensor_mul(
                                    pt[:, i, 0:128], pt[:, i, 0:128], msk_sb[:, 0:128]
                                )
                        emit_av(kb, pt, vw)
                    # normalize: out = num * (1/colsum), colsum broadcast via PE
                    ou = work.tile([65, 1024], bf16, name="ou", tag="ou", bufs=2)
                    for i in range(2):
                        nc.any.tensor_copy(
                            out=ou[:, 512 * i : 512 * (i + 1)], in_=pav[i][:]
                        )
                    at = work.tile([64, 1024], bf16, name="at", tag="at")
                    for i in range(2):
                        pb = psP.tile([64, 512], f32, name=f"pb{i}", tag="pp")
                        nc.tensor.matmul(
                            pb[:],
                            ones_sb[64:65, :],
                            ou[64:65, 512 * i : 512 * (i + 1)],
                            start=True,
                            stop=True,
                        )
                        rbc = work.tile([64, 512], f32, name="rbc", tag="rbc", bufs=2)
                        nc.vector.reciprocal_approx_fast(out=rbc[:], in_=pb[:])
                        # pure-SBUF multiply: run on the otherwise-idle GpSimd
                        # so VectorE (PSUM drains, masks, rope) stops gating PE
                        nc.gpsimd.tensor_mul(
                            at[:, 512 * i : 512 * (i + 1)],
                            ou[0:64, 512 * i : 512 * (i + 1)],
                            rbc[:],
                        )
                    for i in range(2):
                        if qb == QB - 1:
                            dst = cc3_in[pidx][64 * i : 64 * (i + 1), :]
                        else:
                            dst = cc_in[qb][
                                128 * pidx + 64 * i : 128 * pidx + 64 * (i + 1), :
                            ]
                        nc.sync.dma_start(
                            out=dst, in_=at[:, 512 * i : 512 * (i + 1)]
                        )

                def _ag(tin, tout, rows):
                    if NO_CC:
                        nc.sync.dma_start(out=tout[0:rows, :], in_=tin[:, :])
                    else:
                        nc.gpsimd.collective_compute(
                            "AllGather",
                            mybir.AluOpType.bypass,
                            replica_groups=RG,
                            ins=[tin.opt()],
                            outs=[tout.opt()],
                        )

                def allgather(qb):
                    _ag(cc_in[qb], cc_out[qb], 512)

                # ---------------- o_proj quanta ----------------
                def oproj_quanta(qb):
                    cct = {}
                    last = qb == QB - 1
                    # for the per-pair-gathered last block, order contraction
                    # chunks pair-major so pair 3's chunks accumulate last
                    hc_order = (
                        [4 * c + p for p in range(4) for c in range(4)]
                        if last
                        else list(range(HC))
                    )

                    def load(lo, hi):
                        def _f():
                            for hc in hc_order[lo:hi]:
                                t = work.tile(
                                    [128, 512], bf16, name=f"cct{hc}", tag=f"cct{hc}",
                                    bufs=2,
                                )
                                if last:
                                    c, p = divmod(hc, 4)
                                    src = cc3_out[p][128 * c : 128 * (c + 1), :]
                                else:
                                    src = cc_out[qb][128 * hc : 128 * (hc + 1), :]
                                nc.sync.dma_start(out=t[:], in_=src)
                                cct[hc] = t
                        return _f

                    def rb_quant(rb):
                        def _f():
                            po = psP.tile([128, DOUT], f32, name="po", tag="pp")
                            for n, hc in enumerate(hc_order):
                                nc.tensor.matmul(
                                    po[:],
                                    cct[hc][:, 128 * rb : 128 * (rb + 1)],
                                    wo_t[hc][:],
                                    start=(n == 0),
                                    stop=(n == HC - 1),
                                )
                            ot = work.tile([128, DOUT], f32, name="ot", tag="ot", bufs=2)
                            nc.any.tensor_copy(out=ot[:], in_=po[:])
                            nc.sync.dma_start(
                                out=out_d[
                                    512 * qb + 128 * rb : 512 * qb + 128 * (rb + 1), :
                                ],
                                in_=ot[:],
                            )
                        return _f

                    return [load(0, 8), load(8, 16)] + [rb_quant(rb) for rb in range(4)]

                # ---------------- the interleaved schedule ----------------
                # PE stream follows x-window DMA arrival: w0/w1 work first,
                # w2/w3-dependent V blocks and K c2=1 after
                for kb in range(8):
                    vproj(kb)
                proj_rope(wk8, 0, None, 0, dest_split=kTd)
                # dup each K chunk right away so the copy overlaps later work
                for h in range(2):
                    nc.sync.dma_start(
                        out=kTd[h][64:128, 0:CH], in_=kTd[h][0:64, 0:CH]
                    )
                for p in range(4):
                    proj_rope(wq8, 128 * p, qT[p], 0)
                for kb in range(8, 12):
                    vproj(kb)
                proj_rope(wk8, 0, None, 1, dest_split=kTd)
                for h in range(2):
                    nc.sync.dma_start(
                        out=kTd[h][64:128, CH : 2 * CH], in_=kTd[h][0:64, CH : 2 * CH]
                    )
                for kb in range(12, KB):
                    vproj(kb)

                oq = {}
                # injections: (qb, pidx) -> list of emission closures
                inj = {
                    (0, 1): [lambda: proj_rope(wq8, 0, qT[0], 1)],
                    (0, 3): [lambda: proj_rope(wq8, 128, qT[1], 1)],
                    (1, 1): [lambda: proj_rope(wq8, 256, qT[2], 1)],
                    (1, 3): [lambda: proj_rope(wq8, 384, qT[3], 1)],
                }

                for qb in range(QB):
                    if qb == 2:
                        oq[0] = oproj_quanta(0)
                        inj[(2, 0)] = [oq[0][0]]
                        inj[(2, 1)] = [oq[0][1], oq[0][2]]
                        inj[(2, 2)] = [oq[0][3], oq[0][4]]
                        inj[(2, 3)] = [oq[0][5]]
                    if qb == 3:
                        oq[1] = oproj_quanta(1)
                        oq[2] = oproj_quanta(2)
                        inj[(3, 0)] = oq[1][0:3]
                        inj[(3, 1)] = oq[1][3:6]
                        inj[(3, 2)] = oq[2][0:3]
                        # keep two oproj(2) quanta for the tail so PE has
                        # AG3p3-independent work while that gather is in flight
                        inj[(3, 3)] = oq[2][3:4]
                    for pidx in range(4):
                        attn_pair(qb, pidx)
                        if qb == QB - 1:
                            _ag(cc3_in[pidx], cc3_out[pidx], 128)
                        for f in inj.get((qb, pidx), []):
                            f()
                    if qb < QB - 1:
                        allgather(qb)
                oq3 = oproj_quanta(3)
                # tail order: finish oproj(2) first so its cct slots free up
                # for qb3's loads, which then overlap the AG3p3 wait
                for f in oq[2][4:6] + oq3:
                    f()

            if bench_iters:
                with tc.For_i(0, bench_iters, 1, name="bench"):
                    emit_body()
            else:
                emit_body()

    nc.compile()
    return nc


def prep_inputs(x, cos, sin, wq, wk, wv, wo):
    """Shard + reformat full inputs into per-core input maps.

    (fp8 projections were tried and reverted: attention output is a
    random-walk sum over keys, so e4m3's ~2.6% RMS quantization error lands
    ~1:1 on the output -- 2.8-5.4% rel err, over the 2e-2 gate.)
    """
    bf = ml_dtypes.bfloat16
    b, s, d = x.shape
    dout = d // 4
    cos2 = np.tile(np.ascontiguousarray(cos.T), (2, 1)).astype(bf)
    sinT = np.ascontiguousarray(sin.T)
    sinsw = np.concatenate([-sinT[:32], sinT[32:]], axis=0)
    sinsw2 = np.tile(sinsw, (2, 1)).astype(bf)
    # rotate-half permutation: tmp[i] = raw[sigma(i)]; out = R.T @ raw
    rotm = np.zeros((128, 128), np.float32)
    for i in range(128):
        j = (i // 64) * 64 + ((i % 64) + 32) % 64
        rotm[j, i] = 1.0
    rotm = rotm.astype(bf)
    k_loc = np.arange(128)[:, None]
    q_loc = np.arange(128)[None, :]
    masks = (k_loc <= q_loc).astype(np.float32).astype(bf)  # [128,128]

    dc = d // 128
    in_maps = []
    for c in range(N_CORES):
        bb, g = divmod(c, 4)
        in_maps.append(
            {
                "xT": np.ascontiguousarray(x[bb].T.reshape(dc, 128, s)).astype(bf),
                "wq": np.ascontiguousarray(
                    wq[:, 512 * g : 512 * (g + 1)].reshape(dc, 128, 512)
                ).astype(bf),
                "wk": np.ascontiguousarray(
                    wk[:, 128 * g : 128 * (g + 1)].reshape(dc, 128, 128)
                ).astype(bf),
                "wv": np.ascontiguousarray(
                    wv[:, 128 * g : 128 * (g + 1)].reshape(dc, 128, 128)
                ).astype(bf),
                "wo": np.ascontiguousarray(wo[:, dout * g : dout * (g + 1)]).astype(bf),
                "cos2": cos2,
                "sinsw2": sinsw2,
                "rot": rotm,
                "masks": masks,
            }
        )
    return in_maps


def assemble_output(results, b, s, d):
    full = np.empty((b, s, d), np.float32)
    dout = d // 4
    for c in range(N_CORES):
        bb, g = divmod(c, 4)
        full[bb][:, dout * g : dout * (g + 1)] = results[c]["out"]
    return full


def kernel(**inputs):
    x = np.asarray(inputs["x"], np.float32)
    b, s, d = x.shape
    key = (s, d)
    if key not in _cache:
        _cache[key] = build_program(S=s, D=d)
    nc = _cache[key]
    in_maps = prep_inputs(
        x,
        np.asarray(inputs["cos"], np.float32),
        np.asarray(inputs["sin"], np.float32),
        np.asarray(inputs["wq"], np.float32),
        np.asarray(inputs["wk"], np.float32),
        np.asarray(inputs["wv"], np.float32),
        np.asarray(inputs["wo"], np.float32),
    )
    from concourse.bass_utils import run_bass_kernel_spmd

    res = run_bass_kernel_spmd(nc, in_maps, core_ids=list(range(N_CORES)))
    return assemble_output(res.results, b, s, d)
